# revision 9
# baseline (speedup 1.0000x reference)
"""Dynamic 3x3 per-pixel filter (DynamicFilterLayer2D) on 8 Trainium2 cores.

Reference: out[b,c,h,w] = sum_{i,j in 3x3} xpad[b,c,h+i,w+j] * f[b,c,(3i+j),h,w]

Sharding: H is split into 8 bands of 32 rows; each core processes all
(b, c) images for its band (data parallel, 1-row halo). Per-core layout:
partitions = 128 (b,c) images (2 groups of 128), free dim = flat pixels.

All HBM traffic is fp16 (the 2e-2 rel-err gate leaves ~20x margin), which
halves DMA bytes versus fp32; per-core DMA roofline is ~371 GB/s.

Engine split (rates measured on HW):
- DVE in 2x fp16 mode (0.59 ns/elem): two fused multiplies produce the 9
  per-tap product planes (the 3x3 window shift is just an offset into the
  row-flat x tile, so every AP is packed-contiguous and 2x-eligible),
  then 2 adds fold the row-0 planes into one partial.
- PE sums the 6 planes of rows i=1,2 plus that partial into PSUM via
  identity-weight matmuls (PSUM accumulation), chunked per 512-col bank;
  PSUM is managed as 4 x 2-bank tiles for finer recycling.
- ACT copies PSUM (fp32) to the fp16 out tile and issues the out DMA.
The final super skips PE and folds everything on DVE so the kernel tail
is not serialized behind the PE -> ACT -> DMA chain.
GPSIMD tensor ops share SBUF ports with DVE (measured ~4x slowdown when
co-active) and are not used.

Filter border columns (taps that would read x column padding) are zeroed
host-side, so column wrap reads multiply garbage by 0 and no x column
padding is needed.
"""

import numpy as np

B, C, H, W = 8, 32, 256, 256
K = 3
N_CORES = 8
BAND = H // N_CORES            # 32 rows per core
RD = 8                         # rows per full super-tile
FS = RD * W                    # pixels per partition per super-tile (2048)
N_IMG = B * C                  # 256 images
P = 128
GROUPS = N_IMG // P            # 2
XLEN = (BAND + 2) * W + 2      # per-image padded x row storage (8706)
PSUM_CHUNK = 512               # fp32 elems per PSUM bank per partition
PSUM_TILE = 2 * PSUM_CHUNK     # 2-bank psum tiles (1024 cols)

_CACHE = {}


def _strided_ap(tile_ap, dims, offset):
    """Copy of tile_ap with free dims replaced by [[step, count], ...]
    (element units) at element offset; partition dim preserved."""
    import bass_rust
    c = tile_ap.copy()
    part = list(c.ap)[0]
    c.ap = bass_rust.VecI64Pair([list(part)] + [list(d) for d in dims])
    c.offset = offset
    return c


def _build_module():
    import concourse.bacc as bacc
    import concourse.mybir as mybir
    from concourse.tile import TileContext

    fp16 = mybir.dt.float16
    fp32 = mybir.dt.float32
    add = mybir.AluOpType.add
    mult = mybir.AluOpType.mult

    nc = bacc.Bacc("TRN2", target_bir_lowering=False, debug=False)
    x_d = nc.dram_tensor("x_s", [N_IMG, XLEN], fp16,
                         kind="ExternalInput").ap()
    # planar taps: [img, tap, band_row, w]
    f_d = nc.dram_tensor("f_s", [N_IMG, K * K, BAND, W], fp16,
                         kind="ExternalInput").ap()
    eye_d = nc.dram_tensor("eye_s", [P, P], fp16, kind="ExternalInput").ap()
    o_d = nc.dram_tensor("o_s", [N_IMG, BAND, W], fp16,
                         kind="ExternalOutput").ap()

    # tiny leading supers shorten the initial f/x-DMA ramp before the
    # DVE can start; small trailing supers shorten the drain tail.
    # Small supers draw from their own f pool so prefetch of the big
    # supers is not blocked on buffer recycling.
    supers = {0: [(0, 1), (1, 1), (2, 2), (4, 4), (RD, RD), (2 * RD, RD),
                  (3 * RD, RD)],
              1: [(0, RD), (RD, RD), (2 * RD, RD), (3 * RD, RD // 2),
                  (3 * RD + RD // 2, RD // 2)]}
    last = (1, supers[1][-1][0])

    with TileContext(nc) as tc:
        with (
            tc.tile_pool(name="ey", bufs=1) as epool,
            tc.tile_pool(name="xp", bufs=3) as xpool,
            tc.tile_pool(name="fb", bufs=2) as fbig,
            tc.tile_pool(name="fs", bufs=2) as fsmall,
            tc.tile_pool(name="pp", bufs=1) as ppool,
            tc.tile_pool(name="p6", bufs=2) as p6pool,
            tc.tile_pool(name="st", bufs=2) as spool,
            tc.tile_pool(name="op", bufs=2) as opool,
            tc.tile_pool(name="ps", bufs=4, space="PSUM") as psumpool,
        ):
            eye = epool.tile([P, P], fp16, tag="eye")
            nc.scalar.dma_start(out=eye[:, :], in_=eye_d[:, :])
            for g in range(GROUPS):
                p0 = g * P
                for (r0, rd) in supers[g]:
                    fs = rd * W
                    xlen = (rd + 2) * W + 2
                    fpool = fsmall if rd <= 2 else fbig
                    ft = fpool.tile([P, K * K * W * (2 if rd <= 2 else RD)],
                                    fp16, tag="f")
                    nc.sync.dma_start(
                        out=ft[:, 0:K * K * fs],
                        in_=f_d[p0:p0 + P, :, r0:r0 + rd, :],
                    )
                    xt = xpool.tile([P, (RD + 2) * W + 2], fp16, tag="x")
                    nc.scalar.dma_start(
                        out=xt[:, 0:xlen],
                        in_=x_d[p0:p0 + P, r0 * W:r0 * W + xlen],
                    )
                    # products: taps 3-8 (rows i=1,2) into p6 first so the
                    # PE can start accumulating ASAP; taps 0-2 into pt
                    pt = ppool.tile([P, 3 * FS], fp16, tag="p")
                    p6 = p6pool.tile([P, 6 * FS], fp16, tag="q")
                    xinB = _strided_ap(xt[:, :], [[W, 2], [1, K], [1, fs]], W)
                    finB = _strided_ap(ft[:, :], [[K * fs, 2], [fs, K],
                                                  [1, fs]], K * fs)
                    poutB = _strided_ap(p6[:, :], [[K * fs, 2], [fs, K],
                                                   [1, fs]], 0)
                    nc.vector.tensor_tensor(poutB, xinB, finB, mult)
                    xinA = _strided_ap(xt[:, :], [[1, K], [1, fs]], 0)
                    finA = _strided_ap(ft[:, :], [[fs, K], [1, fs]], 0)
                    poutA = _strided_ap(pt[:, :], [[fs, K], [1, fs]], 0)
                    nc.vector.tensor_tensor(poutA, xinA, finA, mult)
                    is_last = (g, r0) == last
                    if is_last:
                        # fold all 9 planes on DVE: no PE/ACT tail chain
                        nc.vector.tensor_tensor(
                            p6[:, 0:3 * fs], p6[:, 0:3 * fs],
                            p6[:, 3 * fs:6 * fs], add)
                        nc.vector.tensor_tensor(
                            pt[:, 0:fs], pt[:, 0:fs], pt[:, fs:2 * fs], add)
                        nc.vector.tensor_tensor(
                            pt[:, 0:fs], pt[:, 0:fs], pt[:, 2 * fs:3 * fs],
                            add)
                        nc.vector.tensor_tensor(
                            p6[:, 0:fs], p6[:, 0:fs], p6[:, fs:2 * fs], add)
                        nc.vector.tensor_tensor(
                            p6[:, 0:fs], p6[:, 0:fs], p6[:, 2 * fs:3 * fs],
                            add)
                        ot = opool.tile([P, FS], fp16, tag="o")
                        nc.vector.tensor_tensor(
                            ot[:, 0:fs], pt[:, 0:fs], p6[:, 0:fs], add)
                        nc.scalar.dma_start(
                            out=o_d[p0:p0 + P, r0:r0 + rd, :],
                            in_=ot[:, 0:fs],
                        )
                        continue
                    # DVE folds planes 0..2 into st
                    nc.vector.tensor_tensor(
                        pt[:, 0:fs], pt[:, 0:fs], pt[:, fs:2 * fs], add)
                    st = spool.tile([P, FS], fp16, tag="s")
                    nc.vector.tensor_tensor(
                        st[:, 0:fs], pt[:, 0:fs], pt[:, 2 * fs:3 * fs], add)
                    # PE: psum = p3+..+p8 + st, 2-bank psum tiles
                    ot = opool.tile([P, FS], fp16, tag="o")
                    for h0 in range(0, fs, PSUM_TILE):
                        h1 = min(fs, h0 + PSUM_TILE)
                        pst = psumpool.tile([P, PSUM_TILE], fp32, tag="ps")
                        for c0 in range(h0, h1, PSUM_CHUNK):
                            c1 = min(h1, c0 + PSUM_CHUNK)
                            srcs = [p6[:, t * fs + c0:t * fs + c1]
                                    for t in range(6)] + [st[:, c0:c1]]
                            for k, src in enumerate(srcs):
                                nc.tensor.matmul(
                                    out=pst[:, c0 - h0:c1 - h0],
                                    lhsT=eye[:, :], rhs=src,
                                    start=(k == 0), stop=(k == len(srcs) - 1),
                                )
                        # ACT: downcast this psum tile into the out tile
                        nc.scalar.copy(ot[:, h0:h1], pst[:, 0:h1 - h0])
                    nc.scalar.dma_start(
                        out=o_d[p0:p0 + P, r0:r0 + rd, :],
                        in_=ot[:, 0:fs],
                    )
    nc.compile()
    return nc


def _get_module():
    if "nc" not in _CACHE:
        _CACHE["nc"] = _build_module()
    return _CACHE["nc"]


def _shard_inputs(x, dynamic_filters):
    """Per-core input maps. x: [B,C,H,W] f32, filters: [B,C*9,H,W] f32."""
    xp = np.pad(x, ((0, 0), (0, 0), (1, 1), (0, 0))).astype(np.float16)
    # planar taps [img, t, H, W]; zero border cols (j=0 @ w=0, j=2 @ w=W-1)
    fp = np.ascontiguousarray(
        dynamic_filters.reshape(N_IMG, K * K, H, W)).astype(np.float16)
    fp[:, 0::3, :, 0] = 0.0
    fp[:, 2::3, :, W - 1] = 0.0
    eye = np.eye(P, dtype=np.float16)

    in_maps = []
    for n in range(N_CORES):
        r = n * BAND
        xs = xp[:, :, r:r + BAND + 2, :].reshape(N_IMG, (BAND + 2) * W)
        xs_flat = np.zeros((N_IMG, XLEN), np.float16)
        xs_flat[:, 1:-1] = xs
        fs = np.ascontiguousarray(fp[:, :, r:r + BAND])
        in_maps.append({"x_s": xs_flat, "f_s": fs, "eye_s": eye})
    return in_maps


def kernel(x, dynamic_filters, _trace=False):
    from concourse import bass_utils

    x = np.asarray(x, dtype=np.float32)
    dynamic_filters = np.asarray(dynamic_filters, dtype=np.float32)
    nc = _get_module()
    in_maps = _shard_inputs(x, dynamic_filters)
    res = bass_utils.run_bass_kernel_spmd(
        nc, in_maps, list(range(N_CORES)), trace=_trace)
    out = np.concatenate(
        [res.results[n]["o_s"].reshape(B, C, BAND, W).astype(np.float32)
         for n in range(N_CORES)],
        axis=2)
    _CACHE["last_exec_time_ns"] = res.exec_time_ns
    return out
